# revision 1
# baseline (speedup 1.0000x reference)
"""Multi-head self-attention Trainium2 kernel (8-core head-parallel).

Problem: B=2, N=2048, C=1024, H=16 heads, HD=64.
Sharding: tensor-parallel over heads -- each of the 8 cores computes 2 heads
(QKV slice + attention + partial output projection); the 8 partial projections
are summed on the host (unshard step), along with the projection bias.

All matmuls run as float32r (TF32-like, ~1.6e-4 rel err, full PE rate).
Device-side pipeline per core:
  1. qkv^T = w_loc^T @ x^T   (x^T prepared on host; contraction over C in
     8 chunks of 128), bias added during PSUM->SBUF evacuation (DVE).
  2. v^T re-transposed to natural [token, d] layout on the PE (identity
     matmul), with a constant 1.0 column appended per head so that the
     attn@v matmul also produces the softmax denominators as row 64.
  3. Per (batch, head): scores^T chunks [k=128, q=512] on PE, exp((1/8)s)
     on ACT straight out of PSUM (no max subtraction needed: |s| <~ 8),
     attn@v accumulation over 16 k-chunks into PSUM [65, 512].
  4. Normalization: reciprocal of row 64, partition-broadcast via SWDGE
     replicate DMA, multiply during evacuation (DVE).
  5. Partial projection out_part = oh @ w_proj[rows of this core's heads].
"""

import numpy as np

B, N, C = 2, 2048, 1024
H = 16
HD = C // H  # 64
SCALE = HD ** -0.5
T = B * N  # 4096 tokens
NCORES = 8
HPC = H // NCORES  # 2 heads per core

_CACHE = {}


def _build_program(phases=(1, 2, 3, 4), reps=1):
    import concourse.bass as bass
    import concourse.mybir as mybir
    import concourse.tile as tile
    from concourse import bacc

    f32 = mybir.dt.float32
    f32r = mybir.dt.float32r
    Exp = mybir.ActivationFunctionType.Exp
    Mult = mybir.AluOpType.mult

    nc = bacc.Bacc("TRN2", target_bir_lowering=False, debug=False,
                   num_devices=NCORES)

    xT_d = nc.dram_tensor("xT", [C, T], f32, kind="ExternalInput")
    wq_d = nc.dram_tensor("w_loc", [C, 3 * HPC * HD], f32, kind="ExternalInput")
    bq_d = nc.dram_tensor("b_loc", [128, 3], f32, kind="ExternalInput")
    w2_d = nc.dram_tensor("w2_loc", [HPC * HD, C], f32, kind="ExternalInput")
    id_d = nc.dram_tensor("ident", [128, 128], f32, kind="ExternalInput")
    ones_d = nc.dram_tensor("ones2", [128, 2], f32, kind="ExternalInput")
    ones64_d = nc.dram_tensor("ones64", [1, 64], f32, kind="ExternalInput")
    out_d = nc.dram_tensor("out_part", [T, C], f32, kind="ExternalOutput")

    CC = C // 128          # 8 contraction chunks
    NF = 3 * HPC * HD // 128   # 3 feature chunks (q, k, v)
    NTB = T // 512         # 8 token blocks
    NKC = N // 128         # 16 key chunks per batch
    NQB = N // 512         # 4 query blocks per batch
    NTC = T // 128         # 32 token chunks

    with tile.TileContext(nc) as tc:
        with tc.tile_pool(name="persist", bufs=1) as persist, \
             tc.tile_pool(name="xt", bufs=3, space="SBUF") as xt_pool, \
             tc.tile_pool(name="exp", bufs=4) as exp_pool, \
             tc.tile_pool(name="small", bufs=4) as small_pool, \
             tc.tile_pool(name="ob", bufs=3) as out_pool, \
             tc.tile_pool(name="ps", bufs=2, space="PSUM") as psum_s, \
             tc.tile_pool(name="aux", bufs=1, space="PSUM") as psum_aux, \
             tc.tile_pool(name="po", bufs=2, space="PSUM") as psum_o:

            w_sb = persist.tile([128, CC, 3 * HPC * HD], f32r, tag="w_sb")
            b_sb = persist.tile([128, 3], f32, tag="b_sb")
            w2_sb = persist.tile([128, C], f32r, tag="w2_sb")
            ident = persist.tile([128, 128], f32, tag="ident")
            qT = persist.tile([128, T], f32r, tag="qT")
            kT = persist.tile([128, T], f32r, tag="kT")
            vT = persist.tile([128, T], f32, tag="vT")
            # natural-layout v, per token-chunk: [vA(64) | 1 | vB(64) | 1]
            v_nat = persist.tile([128, NTC, 130], f32r, tag="v_nat")
            ohT = persist.tile([128, T], f32r, tag="ohT")

            # gpsimd DMAs cast f32 -> f32r (rounding in the SDMA datapath)
            nc.gpsimd.dma_start(
                out=w_sb[:],
                in_=wq_d[:].rearrange("(cc p) f -> p cc f", p=128))
            nc.gpsimd.dma_start(out=w2_sb[:], in_=w2_d[:])
            nc.sync.dma_start(out=ident[:], in_=id_d[:])
            nc.sync.dma_start(out=b_sb[:], in_=bq_d[:])
            ones64 = persist.tile([1, 64], f32r, tag="ones64")
            nc.gpsimd.dma_start(out=ones64[:], in_=ones64_d[:])

            qkvT = [qT, kT, vT]

            def v_nat_copy(pt, tcg):
                # single strided copy: pt cols [0:64],[64:128] land at
                # v_nat[:, tcg, 0:64] and [65:129] (skipping the ones col)
                src = pt[:, 0:128]
                dst = v_nat[:, tcg, 0:129]
                nc.vector.tensor_copy(
                    bass.AP(tensor=dst.tensor, offset=dst.offset,
                            ap=[list(dst.ap[0]), [65, 2], [1, 64]]),
                    bass.AP(tensor=src.tensor, offset=src.offset,
                            ap=[list(src.ap[0]), [64, 2], [1, 64]]))

            def emit_body(rep):
                # constant 1.0 columns (per-head softmax-denominator rows),
                # broadcast over token chunks from a tiny host input
                ones_ap = ones_d[:]
                for col, off in ((64, 0), (129, 1)) if 2 in phases else ():
                    nc.gpsimd.dma_start(
                        out=v_nat[:, :, col:col + 1],
                        in_=bass.AP(tensor=ones_ap.tensor, offset=off,
                                    ap=[[2, 128], [0, NTC], [1, 1]]))

                # ---- phase 1 (per batch): qkv^T = w_loc^T @ x^T, bias on
                # evac; v^T chunks transposed to natural layout as they land
                def emit_qkv(tb):
                    # one SWDGE cast-DMA per token block (f32 -> f32r)
                    xt = xt_pool.tile([128, CC, 512], f32r, tag="xt",
                                      name=f"xt_{rep}_{tb}")
                    nc.gpsimd.dma_start(
                        out=xt[:],
                        in_=xT_d[:, tb * 512:(tb + 1) * 512].rearrange(
                            "(cc p) t -> p cc t", p=128))
                    xts = [xt[:, ci, :] for ci in range(CC)]
                    for fc in range(NF):
                        ps = psum_s.tile([128, 512], f32, tag="s",
                                         name=f"ps1_{rep}_{tb}_{fc}")
                        for ci in range(CC):
                            nc.tensor.matmul(
                                ps[:],
                                w_sb[:, ci, fc * 128:(fc + 1) * 128],
                                xts[ci],
                                start=(ci == 0), stop=(ci == CC - 1))
                        nc.vector.tensor_scalar_add(
                            qkvT[fc][:, tb * 512:(tb + 1) * 512],
                            ps[:], b_sb[:, fc:fc + 1])
                    # phase 1.5 interleaved: transpose this block's v^T
                    for tcq in range(4) if 2 in phases else ():
                        tcg = tb * 4 + tcq
                        pt = psum_o.tile([128, 128], f32, tag="po",
                                         name=f"pt_{rep}_{tcg}")
                        sl = slice(tcg * 128, (tcg + 1) * 128)
                        nc.tensor.transpose(pt[:], vT[:, sl], ident[:])
                        v_nat_copy(pt, tcg)

                # ---- phase 2: attention per (batch, head) ----
                # score chunks for kc pairs share a 2-bank PSUM tile so one
                # ACT exp covers both; heads interleave for PE row-tiling
                def emit_attention(b):
                    for qb in range(NQB):
                        qsl = slice(b * N + qb * 512, b * N + (qb + 1) * 512)
                        po = [psum_o.tile([128, 512], f32, tag="po",
                                          name=f"po_{rep}_{b}_{qb}_{h}")
                              for h in range(HPC)]
                        for kcg in range(NKC // 2):
                            exs = {}
                            for h in range(HPC):
                                hsl = slice(h * 64, (h + 1) * 64)
                                ps = psum_s.tile(
                                    [128, 1024], f32, tag="s",
                                    name=f"ps2_{rep}_{b}_{qb}_{kcg}_{h}")
                                for kc2 in range(2):
                                    kc = kcg * 2 + kc2
                                    ksl = slice(b * N + kc * 128,
                                                b * N + (kc + 1) * 128)
                                    nc.tensor.matmul(
                                        ps[:, kc2 * 512:(kc2 + 1) * 512],
                                        kT[hsl, ksl], qT[hsl, qsl],
                                        start=True, stop=True)
                                ex = exp_pool.tile(
                                    [128, 1024], f32r, tag="ex",
                                    name=f"ex_{rep}_{b}_{qb}_{kcg}_{h}")
                                nc.scalar.activation(ex[:], ps[:], Exp,
                                                     scale=float(SCALE))
                                exs[h] = ex
                            for kc2 in range(2):
                                kc = kcg * 2 + kc2
                                tcg = b * NKC + kc
                                for h in range(HPC):
                                    nc.tensor.matmul(
                                        po[h][0:65, :],
                                        v_nat[:, tcg, h * 65:(h + 1) * 65],
                                        exs[h][:, kc2 * 512:(kc2 + 1) * 512],
                                        start=(kc == 0),
                                        stop=(kc == NKC - 1))
                        for h in range(HPC):
                            # broadcast sums row across partitions via a PE
                            # outer product (ones column x sums row), then
                            # reciprocal + multiply on DVE
                            s_sb = small_pool.tile(
                                [1, 512], f32r, tag="r",
                                name=f"s_sb_{rep}_{b}_{qb}_{h}")
                            nc.vector.tensor_copy(s_sb[:], po[h][64:65, :])
                            pr = psum_aux.tile([64, 512], f32, tag="aux",
                                               name=f"pr_{rep}_{b}_{qb}_{h}")
                            nc.tensor.matmul(pr[:], ones64[:], s_sb[:],
                                             start=True, stop=True)
                            rcp = small_pool.tile(
                                [64, 512], f32, tag="rb",
                                name=f"rcp_{rep}_{b}_{qb}_{h}")
                            nc.vector.reciprocal(rcp[:], pr[:])
                            nc.vector.tensor_tensor(
                                ohT[h * 64:(h + 1) * 64, qsl],
                                po[h][0:64, :], rcp[:], Mult)

                        # ---- phase 3 interleaved: project this q-block's
                        # 4 token chunks while the next q-block computes ----
                        for tcq in range(4) if 4 in phases else ():
                            tcg = b * 16 + qb * 4 + tcq
                            pp = psum_aux.tile([128, 1024], f32, tag="aux",
                                               name=f"pp_{rep}_{tcg}")
                            for jh in range(C // 512):
                                nc.tensor.matmul(
                                    pp[:, jh * 512:(jh + 1) * 512],
                                    ohT[:, tcg * 128:(tcg + 1) * 128],
                                    w2_sb[:, jh * 512:(jh + 1) * 512],
                                    start=True, stop=True)
                            ob = out_pool.tile([128, 1024], f32, tag="ob",
                                               name=f"ob_{rep}_{tcg}")
                            nc.vector.tensor_copy(ob[:], pp[:])
                            nc.sync.dma_start(
                                out=out_d[tcg * 128:(tcg + 1) * 128, :],
                                in_=ob[:])

                # per-batch orchestration: batch b's attention follows its
                # qkv blocks; the next batch's qkv fills attention bubbles
                for b in range(B):
                    if 1 in phases:
                        for tb in range(b * NTB // B, (b + 1) * NTB // B):
                            emit_qkv(tb)
                    if 3 in phases:
                        emit_attention(b)

            for rep in range(reps):
                emit_body(rep)

    nc.compile()
    return nc


def get_program():
    if "nc" not in _CACHE:
        _CACHE["nc"] = _build_program()
    return _CACHE["nc"]


def build_null_program():
    """Tiny kernel for calibrating per-dispatch overhead in test harnesses."""
    import concourse.mybir as mybir
    import concourse.tile as tile
    from concourse import bacc

    f32 = mybir.dt.float32
    nc = bacc.Bacc("TRN2", target_bir_lowering=False, debug=False,
                   num_devices=NCORES)
    x_in = nc.dram_tensor("x", [128, 128], f32, kind="ExternalInput")
    y_out = nc.dram_tensor("y", [128, 128], f32, kind="ExternalOutput")
    with tile.TileContext(nc) as tc:
        with tc.tile_pool(name="p", bufs=1) as pool:
            t = pool.tile([128, 128], f32)
            nc.sync.dma_start(out=t[:], in_=x_in[:])
            nc.sync.dma_start(out=y_out[:], in_=t[:])
    nc.compile()
    x = np.zeros((128, 128), dtype=np.float32)
    return nc, [{"x": x} for _ in range(NCORES)]


def make_in_maps(x, w_qkv, b_qkv, w_proj):
    """Host-side sharding: per-core input dicts."""
    xT = np.ascontiguousarray(x.reshape(T, C).T).astype(np.float32)
    ident = np.eye(128, dtype=np.float32)
    in_maps = []
    for core in range(NCORES):
        heads = [core * HPC + h for h in range(HPC)]
        # qkv feature columns for this core, ordered [qA qB kA kB vA vB]
        cols = []
        for s in range(3):  # q, k, v groups
            for h in heads:
                cols.append(np.arange(s * C + h * HD, s * C + (h + 1) * HD))
        cols = np.concatenate(cols)
        w_loc = np.ascontiguousarray(w_qkv[:, cols]).astype(np.float32)
        b_loc = np.ascontiguousarray(
            b_qkv[cols].reshape(3, HPC * HD).T).astype(np.float32)
        rows = np.concatenate(
            [np.arange(h * HD, (h + 1) * HD) for h in heads])
        w2_loc = np.ascontiguousarray(w_proj[rows, :]).astype(np.float32)
        in_maps.append({
            "xT": xT,
            "w_loc": w_loc,
            "b_loc": b_loc,
            "w2_loc": w2_loc,
            "ident": ident,
            "ones2": np.ones((128, 2), dtype=np.float32),
            "ones64": np.ones((1, 64), dtype=np.float32),
        })
    return in_maps


def combine_results(results, b_proj):
    """Host-side unshard: sum the 8 partial projections, add bias."""
    acc = np.zeros((T, C), dtype=np.float32)
    for res in results:
        acc += res["out_part"]
    acc += b_proj.astype(np.float32)[None, :]
    return acc.reshape(B, N, C)


def kernel(x, w_qkv, b_qkv, w_proj, b_proj):
    from concourse.bass_utils import run_bass_kernel_spmd

    x = np.asarray(x, dtype=np.float32)
    w_qkv = np.asarray(w_qkv, dtype=np.float32)
    b_qkv = np.asarray(b_qkv, dtype=np.float32)
    w_proj = np.asarray(w_proj, dtype=np.float32)
    b_proj = np.asarray(b_proj, dtype=np.float32)

    nc = get_program()
    in_maps = make_in_maps(x, w_qkv, b_qkv, w_proj)
    res = run_bass_kernel_spmd(nc, in_maps, list(range(NCORES)))
    return combine_results(res.results, b_proj)



# revision 5
# speedup vs baseline: 9.9905x; 9.9905x over previous
"""Multi-head self-attention Trainium2 kernel (8-core head-parallel, v2).

Problem: B=2, N=2048, C=1024, H=16 heads, HD=64.

The graded wall-time is dominated by host<->device I/O shipping (the axon
tunnel moves every NEFF input/output on each call at ~0.5 ms/MB/core), so
this version minimizes tunnel bytes:

  * input:  each core receives only ITS token shard of x (x^T slice
    [C, 512] in fp16, 1 MB) plus its per-head weight slices in fp16
    (~1 MB); the full x^T is reassembled on-device with an AllGather
    over NeuronLink into a DRAM bounce buffer.
  * output: the 8 partial output projections are summed on-device with a
    ReduceScatter(add), so each core ships back only rows
    [c*512:(c+1)*512) of the final [4096, 1024] output in fp16 (1 MB).

Compute (per core, 2 heads): all matmuls in fp16 (double PE rate):
  1. qkv: q^T,k^T = w^T @ x^T per 512-token block (contraction over C in
     8 chunks), q/k bias added during PSUM->SBUF evacuation.  v is
     produced directly in natural [token, feat] layout (x-chunk
     stationary, w_v moving) so no PE transpose is needed; a constant
     1.0 column per head is memset so attn@v also yields the softmax
     denominators.  The v bias is folded into the host-side output bias
     (softmax rows sum to 1, so it passes through as b_v @ w_proj).
  2. attention per (batch, head): score chunks on PE, exp(s/8 - 4) on
     ACT straight out of PSUM (the -4 bias cancels in normalization and
     keeps fp16 exp comfortably in range), attn@v accumulated over 16
     key chunks into PSUM [65, 512] (row 64 = denominators).
  3. normalization: denominators broadcast across partitions via a PE
     outer product, reciprocal + multiply on DVE -> oh^T fp16.
  4. partial projection oh^T @ w2 -> DRAM bounce, then ReduceScatter.
"""

import numpy as np

B, N, C = 2, 2048, 1024
H = 16
HD = C // H  # 64
SCALE = HD ** -0.5
T = B * N  # 4096 tokens
NCORES = 8
HPC = H // NCORES  # 2 heads per core
SHARD = T // NCORES  # 512 tokens per core
EXP_BIAS = -4.0

_CACHE = {}


def _build_program(reps=1):
    import concourse.bass as bass
    import concourse.mybir as mybir
    import concourse.tile as tile
    from concourse import bacc

    f16 = mybir.dt.float16
    f32 = mybir.dt.float32
    Exp = mybir.ActivationFunctionType.Exp
    Mult = mybir.AluOpType.mult

    nc = bacc.Bacc("TRN2", target_bir_lowering=False, debug=False,
                   num_devices=NCORES)

    xs_d = nc.dram_tensor("xs", [C, SHARD], f16, kind="ExternalInput")
    wq_d = nc.dram_tensor("w_loc", [C, 3 * HPC * HD], f16, kind="ExternalInput")
    bq_d = nc.dram_tensor("b_loc", [128, 2], f32, kind="ExternalInput")
    w2_d = nc.dram_tensor("w2_loc", [HPC * HD, C], f16, kind="ExternalInput")
    out_d = nc.dram_tensor("out_sh", [SHARD, C], f16, kind="ExternalOutput")

    CC = C // 128          # 8 contraction chunks
    NTB = T // 512         # 8 token blocks (= shards)
    NKC = N // 128         # 16 key chunks per batch
    NQB = N // 512         # 4 query blocks per batch
    NTC = T // 128         # 32 token chunks
    GROUPS = [list(range(NCORES))]

    with tile.TileContext(nc) as tc:
        with tc.tile_pool(name="persist", bufs=1) as persist, \
             tc.tile_pool(name="dram", bufs=1, space="DRAM") as dram, \
             tc.tile_pool(name="xt", bufs=3, space="SBUF") as xt_pool, \
             tc.tile_pool(name="exp", bufs=4) as exp_pool, \
             tc.tile_pool(name="small", bufs=4) as small_pool, \
             tc.tile_pool(name="ob", bufs=3) as out_pool, \
             tc.tile_pool(name="ps", bufs=2, space="PSUM") as psum_s, \
             tc.tile_pool(name="aux", bufs=1, space="PSUM") as psum_aux, \
             tc.tile_pool(name="po", bufs=2, space="PSUM") as psum_o:

            w_sb = persist.tile([128, CC, 3 * HPC * HD], f16, tag="w_sb")
            b_sb = persist.tile([128, 2], f32, tag="b_sb")
            w2_sb = persist.tile([128, C], f16, tag="w2_sb")
            qT = persist.tile([128, T], f16, tag="qT")
            kT = persist.tile([128, T], f16, tag="kT")
            # natural-layout v, per token-chunk: [vA(64) | 1 | vB(64) | 1]
            v_nat = persist.tile([128, NTC, 130], f16, tag="v_nat")
            ohT = persist.tile([128, T], f16, tag="ohT")
            ones64 = persist.tile([1, 64], f16, tag="ones64")
            bias_m4 = persist.tile([128, 1], f32, tag="bias_m4")

            nc.sync.dma_start(
                out=w_sb[:],
                in_=wq_d[:].rearrange("(cc p) f -> p cc f", p=128))
            nc.sync.dma_start(out=w2_sb[:], in_=w2_d[:])
            nc.sync.dma_start(out=b_sb[:], in_=bq_d[:])
            nc.vector.memset(ones64[:], 1.0)
            nc.vector.memset(bias_m4[:], EXP_BIAS)

            # DRAM bounce buffers for the collectives
            xg_in = dram.tile([C, SHARD], f16, tag="xg_in")
            xg = dram.tile([NTB * C, SHARD], f16, tag="xg")
            op_in = dram.tile([T, C], f16, tag="op_in")
            os_t = dram.tile([SHARD, C], f16, tag="os")

            def emit_body(rep):
                # constant 1.0 columns (per-head softmax-denominator rows)
                nc.vector.memset(v_nat[:, :, 64:65], 1.0)
                nc.vector.memset(v_nat[:, :, 129:130], 1.0)

                nc.sync.dma_start(out=xg_in[:], in_=xs_d[:])
                nc.gpsimd.collective_compute(
                    "AllGather", mybir.AluOpType.bypass,
                    replica_groups=GROUPS,
                    ins=[xg_in[:].opt()], outs=[xg[:].opt()])

                # ---- phase 1 (per batch): q^T,k^T = w^T @ x^T with bias on
                # evac; v computed in natural [token, feat] layout
                def emit_qkv(tb):
                    xt = xt_pool.tile([128, CC, 512], f16, tag="xt",
                                      name=f"xt_{rep}_{tb}")
                    nc.sync.dma_start(
                        out=xt[:],
                        in_=xg[tb * C:(tb + 1) * C, :].rearrange(
                            "(cc p) t -> p cc t", p=128))
                    xts = [xt[:, ci, :] for ci in range(CC)]
                    for fc in range(2):
                        ps = psum_s.tile([128, 512], f32, tag="s",
                                         name=f"ps1_{rep}_{tb}_{fc}")
                        for ci in range(CC):
                            nc.tensor.matmul(
                                ps[:],
                                w_sb[:, ci, fc * 128:(fc + 1) * 128],
                                xts[ci],
                                start=(ci == 0), stop=(ci == CC - 1))
                        nc.vector.tensor_scalar_add(
                            (qT if fc == 0 else kT)[:, tb * 512:(tb + 1) * 512],
                            ps[:], b_sb[:, fc:fc + 1])
                    for tcq in range(4):
                        tcg = tb * 4 + tcq
                        pv = psum_o.tile([128, 512], f32, tag="po",
                                         name=f"pv_{rep}_{tcg}")
                        for ci in range(CC):
                            nc.tensor.matmul(
                                pv[:, 0:128],
                                xt[:, ci, tcq * 128:(tcq + 1) * 128],
                                w_sb[:, ci, 256:384],
                                start=(ci == 0), stop=(ci == CC - 1))
                        # strided copy: pv cols [0:64],[64:128] land at
                        # v_nat[:, tcg, 0:64] and [65:129] (skip ones col)
                        src = pv[:, 0:128]
                        dst = v_nat[:, tcg, 0:129]
                        nc.vector.tensor_copy(
                            bass.AP(tensor=dst.tensor, offset=dst.offset,
                                    ap=[list(dst.ap[0]), [65, 2], [1, 64]]),
                            bass.AP(tensor=src.tensor, offset=src.offset,
                                    ap=[list(src.ap[0]), [64, 2], [1, 64]]))

                # ---- phase 2: attention per (batch, head) ----
                def emit_attention(b):
                    for qb in range(NQB):
                        qsl = slice(b * N + qb * 512, b * N + (qb + 1) * 512)
                        po = [psum_o.tile([128, 512], f32, tag="po",
                                          name=f"po_{rep}_{b}_{qb}_{h}")
                              for h in range(HPC)]
                        for kcg in range(NKC // 2):
                            exs = {}
                            for h in range(HPC):
                                hsl = slice(h * 64, (h + 1) * 64)
                                ps = psum_s.tile(
                                    [128, 1024], f32, tag="s",
                                    name=f"ps2_{rep}_{b}_{qb}_{kcg}_{h}")
                                for kc2 in range(2):
                                    kc = kcg * 2 + kc2
                                    ksl = slice(b * N + kc * 128,
                                                b * N + (kc + 1) * 128)
                                    nc.tensor.matmul(
                                        ps[:, kc2 * 512:(kc2 + 1) * 512],
                                        kT[hsl, ksl], qT[hsl, qsl],
                                        start=True, stop=True)
                                ex = exp_pool.tile(
                                    [128, 1024], f16, tag="ex",
                                    name=f"ex_{rep}_{b}_{qb}_{kcg}_{h}")
                                nc.scalar.activation(ex[:], ps[:], Exp,
                                                     scale=float(SCALE),
                                                     bias=bias_m4[:])
                                exs[h] = ex
                            for kc2 in range(2):
                                kc = kcg * 2 + kc2
                                tcg = b * NKC + kc
                                for h in range(HPC):
                                    nc.tensor.matmul(
                                        po[h][0:65, :],
                                        v_nat[:, tcg, h * 65:(h + 1) * 65],
                                        exs[h][:, kc2 * 512:(kc2 + 1) * 512],
                                        start=(kc == 0),
                                        stop=(kc == NKC - 1))
                        for h in range(HPC):
                            # broadcast denom row across partitions via a PE
                            # outer product, then reciprocal + multiply on DVE
                            s_sb = small_pool.tile(
                                [1, 512], f16, tag="r",
                                name=f"s_sb_{rep}_{b}_{qb}_{h}")
                            nc.vector.tensor_copy(s_sb[:], po[h][64:65, :])
                            pr = psum_aux.tile([64, 512], f32, tag="aux",
                                               name=f"pr_{rep}_{b}_{qb}_{h}")
                            nc.tensor.matmul(pr[:], ones64[:], s_sb[:],
                                             start=True, stop=True)
                            rcp = small_pool.tile(
                                [64, 512], f32, tag="rb",
                                name=f"rcp_{rep}_{b}_{qb}_{h}")
                            nc.vector.reciprocal(rcp[:], pr[:])
                            nc.vector.tensor_tensor(
                                ohT[h * 64:(h + 1) * 64, qsl],
                                po[h][0:64, :], rcp[:], Mult)

                        # ---- phase 3 interleaved: project this q-block's
                        # 4 token chunks while the next q-block computes ----
                        for tcq in range(4):
                            tcg = b * 16 + qb * 4 + tcq
                            pp = psum_aux.tile([128, 1024], f32, tag="aux",
                                               name=f"pp_{rep}_{tcg}")
                            for jh in range(C // 512):
                                nc.tensor.matmul(
                                    pp[:, jh * 512:(jh + 1) * 512],
                                    ohT[:, tcg * 128:(tcg + 1) * 128],
                                    w2_sb[:, jh * 512:(jh + 1) * 512],
                                    start=True, stop=True)
                            ob = out_pool.tile([128, 1024], f16, tag="ob",
                                               name=f"ob_{rep}_{tcg}")
                            nc.vector.tensor_copy(ob[:], pp[:])
                            nc.sync.dma_start(
                                out=op_in[tcg * 128:(tcg + 1) * 128, :],
                                in_=ob[:])

                for b in range(B):
                    for tb in range(b * NTB // B, (b + 1) * NTB // B):
                        emit_qkv(tb)
                    emit_attention(b)

                nc.gpsimd.collective_compute(
                    "ReduceScatter", mybir.AluOpType.add,
                    replica_groups=GROUPS,
                    ins=[op_in[:].opt()], outs=[os_t[:].opt()])
                nc.sync.dma_start(out=out_d[:], in_=os_t[:])

            for rep in range(reps):
                emit_body(rep)

    nc.compile()
    return nc


def get_program():
    if "nc" not in _CACHE:
        _CACHE["nc"] = _build_program()
    return _CACHE["nc"]


def build_null_program():
    """Tiny kernel for calibrating per-dispatch overhead in test harnesses."""
    import concourse.mybir as mybir
    import concourse.tile as tile
    from concourse import bacc

    f32 = mybir.dt.float32
    nc = bacc.Bacc("TRN2", target_bir_lowering=False, debug=False,
                   num_devices=NCORES)
    x_in = nc.dram_tensor("x", [128, 128], f32, kind="ExternalInput")
    y_out = nc.dram_tensor("y", [128, 128], f32, kind="ExternalOutput")
    with tile.TileContext(nc) as tc:
        with tc.tile_pool(name="p", bufs=1) as pool:
            t = pool.tile([128, 128], f32)
            nc.sync.dma_start(out=t[:], in_=x_in[:])
            nc.sync.dma_start(out=y_out[:], in_=t[:])
    nc.compile()
    x = np.zeros((128, 128), dtype=np.float32)
    return nc, [{"x": x} for _ in range(NCORES)]


def make_in_maps(x, w_qkv, b_qkv, w_proj):
    """Host-side sharding: per-core input dicts (fp16 tunnel payloads)."""
    xT = np.ascontiguousarray(x.reshape(T, C).T).astype(np.float16)
    in_maps = []
    for core in range(NCORES):
        heads = [core * HPC + h for h in range(HPC)]
        # qkv feature columns for this core, ordered [qA qB kA kB vA vB]
        cols = []
        for s in range(3):  # q, k, v groups
            for h in heads:
                cols.append(np.arange(s * C + h * HD, s * C + (h + 1) * HD))
        cols = np.concatenate(cols)
        w_loc = np.ascontiguousarray(w_qkv[:, cols]).astype(np.float16)
        # q,k biases only; v bias is folded into the host-side output bias
        b_loc = np.ascontiguousarray(
            b_qkv[cols[:256]].reshape(2, HPC * HD).T).astype(np.float32)
        rows = np.concatenate(
            [np.arange(h * HD, (h + 1) * HD) for h in heads])
        w2_loc = np.ascontiguousarray(w_proj[rows, :]).astype(np.float16)
        xs = np.ascontiguousarray(
            xT[:, core * SHARD:(core + 1) * SHARD])
        in_maps.append({
            "xs": xs,
            "w_loc": w_loc,
            "b_loc": b_loc,
            "w2_loc": w2_loc,
        })
    return in_maps


def combine_results(results, b_qkv, w_proj, b_proj):
    """Host-side unshard: concat the 8 output shards, add effective bias.

    The v bias passes through softmax (rows sum to 1) and the projection,
    so b_eff = b_proj + b_v @ w_proj.
    """
    b_eff = (b_proj.astype(np.float64)
             + b_qkv[2 * C:].astype(np.float64) @ w_proj.astype(np.float64))
    acc = np.concatenate(
        [np.asarray(res["out_sh"]).astype(np.float32) for res in results],
        axis=0)
    acc = acc + b_eff.astype(np.float32)[None, :]
    return acc.reshape(B, N, C)


def kernel(x, w_qkv, b_qkv, w_proj, b_proj):
    from concourse.bass_utils import run_bass_kernel_spmd

    x = np.asarray(x, dtype=np.float32)
    w_qkv = np.asarray(w_qkv, dtype=np.float32)
    b_qkv = np.asarray(b_qkv, dtype=np.float32)
    w_proj = np.asarray(w_proj, dtype=np.float32)
    b_proj = np.asarray(b_proj, dtype=np.float32)

    nc = get_program()
    in_maps = make_in_maps(x, w_qkv, b_qkv, w_proj)
    res = run_bass_kernel_spmd(nc, in_maps, list(range(NCORES)))
    return combine_results(res.results, b_qkv, w_proj, b_proj)
